# revision 2
# baseline (speedup 1.0000x reference)
"""Trainium2 Bass kernel for AttentionFusionModel (B=4, S=4096, D=200).

out = (attn(x1) + attn(x2)) @ Wo.T + bo, with attn sharing Wq/Wk/Wv.

Sharding: 8 (batch, modality) pairs -> 8 NeuronCores, one full self-attention
per core. Modality fusion = pairwise ReduceScatter between cores (2b, 2b+1),
each core projecting its own attention output first (projection is linear, so
proj(a1 + a2) = proj(a1) + proj(a2); softmax row-normalization commutes with
the projection and is applied post-projection as a per-row scale).

Per-core layout strategy (all big matmuls in bf16, fp32 PSUM accumulate):
  X^T [201, S]   (d on partitions, +ones row)  via PE transposes
  Q^T, K^T [200, S] = (W·sc)^T-stationary matmuls (bias via ones-row aug)
  V [S, 201]     natural layout, +ones column (for sumexp)
  scores^T[k,q] tiles = K^T-slice.T @ Q^T   (contract d: 128+72 blocks)
  expT = Exp(scores^T)  on ScalarE (no max subtraction; |scores| << 80)
  U^T[d+1, q] += V'[k,:].T @ expT           (row 200 = sumexp L)
  proj[q, 201] = U^T-slice.T @ Wo'^T        (col 200 = L passthrough)
  out rows = proj[:, :200] * (1/L) + bo/2
"""

import os
import sys

sys.path.insert(0, "/opt/trn_rl_repo")

import numpy as np
from contextlib import ExitStack

from concourse import bacc, mybir, tile
from concourse.bass_utils import run_bass_kernel_spmd
from concourse.masks import make_identity

F32 = mybir.dt.float32
BF16 = mybir.dt.bfloat16
AF = mybir.ActivationFunctionType
ALU = mybir.AluOpType

B = 4
S = 4096
D = 200
P = 128
D2 = D - P            # 72
DA = D + 1            # 201 (augmented with ones row / sumexp col)
NCORES = 8
RG = [[0, 1], [2, 3], [4, 5], [6, 7]]  # core 2b+m handles (batch b, modality m)

USE_RS = True


def _emit_av(nc, acc1, acc2, v_sb, et, kb, nkb):
    k0 = kb * DA
    st = kb == 0
    sp = kb == nkb - 1
    nc.tensor.matmul(acc1[:], v_sb[:, k0:k0 + P], et[:], start=st, stop=sp)
    nc.tensor.matmul(acc2[:], v_sb[:, k0 + P:k0 + DA], et[:], start=st, stop=sp)


def _emit(ctx, tc, nc, exts, s_len, use_rs):
    x_ext, wq_ext, wk_ext, wv_ext, wo_ext, bo_ext, out_ext, rs_in, rs_out = exts
    QG = min(512, s_len)
    nkb = s_len // P
    nqg = s_len // QG

    pers = ctx.enter_context(tc.tile_pool(name="pers", bufs=1))
    xt0 = pers.tile([P, s_len], BF16)
    xt1 = pers.tile([D2 + 1, s_len], BF16)   # +ones row (row D2)
    qt0 = pers.tile([P, s_len], BF16)
    qt1 = pers.tile([D2, s_len], BF16)
    kt0 = pers.tile([P, s_len], BF16)
    kt1 = pers.tile([D2, s_len], BF16)
    v_sb = pers.tile([P, nkb * DA], BF16)    # 32 tiles of [128, 201], col 200 = 1s
    wq0 = pers.tile([P, D], BF16)
    wq1 = pers.tile([D2 + 1, D], BF16)
    wk0 = pers.tile([P, D], BF16)
    wk1 = pers.tile([D2 + 1, D], BF16)
    wv0 = pers.tile([P, DA], BF16)
    wv1 = pers.tile([D2 + 1, DA], BF16)
    wo0 = pers.tile([P, DA], BF16)
    wo1 = pers.tile([D2 + 1, DA], BF16)
    bo_sb = pers.tile([P, D], F32)
    ident = pers.tile([P, P], F32)

    make_identity(nc, ident[:])
    # ones row lives at partition 72; engine APs need 32-aligned partition
    # bases, so memset [64:73) and let the X^T copies overwrite rows 64..71.
    nc.vector.memset(xt1[64:D2 + 1, :], 1.0)
    nc.sync.dma_start(out=bo_sb[:], in_=bo_ext[:, :])

    # ---- weights: DMA f32, cast to bf16 ----
    with tc.tile_pool(name="wstage", bufs=2) as wst:
        for (ext, b0, b1, width) in [
            (wq_ext, wq0, wq1, D),
            (wk_ext, wk0, wk1, D),
            (wv_ext, wv0, wv1, DA),
            (wo_ext, wo0, wo1, DA),
        ]:
            wf0 = wst.tile([P, DA], F32, tag="wf0")
            wf1 = wst.tile([D2 + 1, DA], F32, tag="wf1")
            nc.sync.dma_start(out=wf0[:, 0:width], in_=ext[0:P, :])
            nc.sync.dma_start(out=wf1[:, 0:width], in_=ext[P:DA, :])
            nc.vector.tensor_copy(b0[:, 0:width], wf0[:, 0:width])
            nc.vector.tensor_copy(b1[:, 0:width], wf1[:, 0:width])

    # ---- phase 1: load X, build X^T (bf16), QKV projections ----
    with ExitStack() as ph1:
        xp = ph1.enter_context(tc.tile_pool(name="xp", bufs=4))
        tps = ph1.enter_context(tc.tile_pool(name="tps", bufs=2, space="PSUM"))
        for n in range(nkb):
            c0, c1 = n * P, (n + 1) * P
            x_in = xp.tile([P, D], F32, tag="xin")
            nc.sync.dma_start(out=x_in[:], in_=x_ext[c0:c1, :])
            p1 = tps.tile([P, P], F32, tag="tp1")
            nc.tensor.transpose(p1[:], x_in[:, 0:P], ident[:])
            p2 = tps.tile([D2, P], F32, tag="tp2")
            nc.tensor.transpose(p2[:], x_in[:, P:D], ident[:])
            nc.vector.tensor_copy(xt0[:, c0:c1], p1[:])
            nc.vector.tensor_copy(xt1[0:D2, c0:c1], p2[:])

        qkps = ph1.enter_context(tc.tile_pool(name="qkps", bufs=2, space="PSUM"))
        CH = min(512, s_len)
        for (w0, w1, t0, t1) in [(wq0, wq1, qt0, qt1), (wk0, wk1, kt0, kt1)]:
            for ob, obw in [(0, P), (1, D2)]:
                tdst = t0 if ob == 0 else t1
                for ch in range(s_len // CH):
                    c0, c1 = ch * CH, (ch + 1) * CH
                    ps = qkps.tile([P, CH], F32, tag="qk")
                    nc.tensor.matmul(ps[0:obw, :], w0[:, ob * P:ob * P + obw],
                                     xt0[:, c0:c1], start=True, stop=False)
                    nc.tensor.matmul(ps[0:obw, :], w1[:, ob * P:ob * P + obw],
                                     xt1[:, c0:c1], start=False, stop=True)
                    nc.vector.tensor_copy(tdst[:, c0:c1], ps[0:obw, :])

        vps = ph1.enter_context(tc.tile_pool(name="vps", bufs=2, space="PSUM"))
        for n in range(nkb):
            c0, c1 = n * P, (n + 1) * P
            pv = vps.tile([P, DA], F32, tag="pv")
            nc.tensor.matmul(pv[:], xt0[:, c0:c1], wv0[:], start=True, stop=False)
            nc.tensor.matmul(pv[:], xt1[:, c0:c1], wv1[:], start=False, stop=True)
            nc.vector.tensor_copy(v_sb[:, n * DA:(n + 1) * DA], pv[:])

    # ---- phase 2: attention + projection + epilogue ----
    with ExitStack() as ph2:
        scp = ph2.enter_context(tc.tile_pool(name="scp", bufs=2, space="PSUM"))
        accp = ph2.enter_context(tc.tile_pool(name="accp", bufs=2, space="PSUM"))
        projp = ph2.enter_context(tc.tile_pool(name="projp", bufs=2, space="PSUM"))
        etp = ph2.enter_context(tc.tile_pool(name="etp", bufs=3))
        utp = ph2.enter_context(tc.tile_pool(name="utp", bufs=2))
        epip = ph2.enter_context(tc.tile_pool(name="epip", bufs=4))
        for qg in range(nqg):
            q0, q1 = qg * QG, (qg + 1) * QG
            acc1 = accp.tile([P, QG], F32, tag="acc1")
            acc2 = accp.tile([D2 + 1, QG], F32, tag="acc2")
            ets = {}
            for kb in range(nkb):
                k0 = kb * P
                sc_ps = scp.tile([P, QG], F32, tag="sc")
                nc.tensor.matmul(sc_ps[:], kt0[:, k0:k0 + P], qt0[:, q0:q1],
                                 start=True, stop=False)
                nc.tensor.matmul(sc_ps[:], kt1[:, k0:k0 + P], qt1[:, q0:q1],
                                 start=False, stop=True)
                et = etp.tile([P, QG], BF16, tag="et")
                nc.scalar.activation(et[:], sc_ps[:], AF.Exp)
                ets[kb] = et
                if kb >= 1:
                    _emit_av(nc, acc1, acc2, v_sb, ets.pop(kb - 1), kb - 1, nkb)
            _emit_av(nc, acc1, acc2, v_sb, ets.pop(nkb - 1), nkb - 1, nkb)

            ut0 = utp.tile([P, QG], BF16, tag="ut0")
            ut1 = utp.tile([D2 + 1, QG], BF16, tag="ut1")
            nc.vector.tensor_copy(ut0[:], acc1[:])
            nc.vector.tensor_copy(ut1[:], acc2[:])
            for qb in range(QG // P):
                pp = projp.tile([P, DA], F32, tag="pp")
                nc.tensor.matmul(pp[:], ut0[:, qb * P:(qb + 1) * P], wo0[:],
                                 start=True, stop=False)
                nc.tensor.matmul(pp[:], ut1[:, qb * P:(qb + 1) * P], wo1[:],
                                 start=False, stop=True)
                rc = epip.tile([P, 1], F32, tag="rc")
                nc.vector.reciprocal(rc[:], pp[:, D:DA])
                ot = epip.tile([P, D], F32, tag="ot")
                nc.vector.tensor_scalar(ot[:], pp[:, 0:D], rc[:], None, ALU.mult)
                nc.vector.tensor_tensor(ot[:], ot[:], bo_sb[:], ALU.add)
                r0 = q0 + qb * P
                dst = rs_in if use_rs else out_ext
                nc.sync.dma_start(out=dst[r0:r0 + P, :], in_=ot[:])

    if use_rs:
        nc.gpsimd.collective_compute(
            "ReduceScatter",
            ALU.add,
            replica_groups=RG,
            ins=[rs_in[:, :].opt()],
            outs=[rs_out[:, :].opt()],
        )
        half = s_len // 2
        nchk = 8 if half % 8 == 0 else 1
        rows = half // nchk
        for i in range(nchk):
            nc.sync.dma_start(out=out_ext[i * rows:(i + 1) * rows, :],
                              in_=rs_out[i * rows:(i + 1) * rows, :])


_CACHE = {}


def _build(s_len=S, use_rs=USE_RS):
    key = (s_len, use_rs)
    if key not in _CACHE:
        nc = bacc.Bacc("TRN2", target_bir_lowering=False, debug=False,
                       num_devices=NCORES)
        x_ext = nc.dram_tensor("x", [s_len, D], F32, kind="ExternalInput")
        wq_ext = nc.dram_tensor("wq", [DA, D], F32, kind="ExternalInput")
        wk_ext = nc.dram_tensor("wk", [DA, D], F32, kind="ExternalInput")
        wv_ext = nc.dram_tensor("wv", [DA, DA], F32, kind="ExternalInput")
        wo_ext = nc.dram_tensor("wo", [DA, DA], F32, kind="ExternalInput")
        bo_ext = nc.dram_tensor("bo", [P, D], F32, kind="ExternalInput")
        out_rows = s_len // 2 if use_rs else s_len
        out_ext = nc.dram_tensor("out", [out_rows, D], F32, kind="ExternalOutput")
        rs_in = rs_out = None
        if use_rs:
            rs_in = nc.dram_tensor("rs_in", [s_len, D], F32)
            rs_out = nc.dram_tensor("rs_out", [out_rows, D], F32)
        exts = (x_ext, wq_ext, wk_ext, wv_ext, wo_ext, bo_ext, out_ext,
                rs_in, rs_out)
        with tile.TileContext(nc) as tc:
            with ExitStack() as ctx:
                _emit(ctx, tc, nc, exts, s_len, use_rs)
        nc.compile()
        _CACHE[key] = nc
    return _CACHE[key]


def _prep_in_maps(m1, m2, Wq, bq, Wk, bk, Wv, bv, Wo, bo, s_len=S):
    sc = np.float32(1.0 / np.sqrt(D))
    wq_p = np.zeros((DA, D), np.float32)
    wq_p[:D] = Wq.T * sc
    wq_p[D] = bq * sc
    wk_p = np.zeros((DA, D), np.float32)
    wk_p[:D] = Wk.T
    wk_p[D] = bk
    wv_p = np.zeros((DA, DA), np.float32)
    wv_p[:D, :D] = Wv.T
    wv_p[D, :D] = bv
    wv_p[D, D] = 1.0
    wo_p = np.zeros((DA, DA), np.float32)
    wo_p[:D, :D] = Wo.T
    wo_p[D, D] = 1.0
    bo_t = np.ascontiguousarray(
        np.broadcast_to((bo * 0.5).astype(np.float32), (P, D)))
    in_maps = []
    for c in range(NCORES):
        b, m = c // 2, c % 2
        x = (m1 if m == 0 else m2)[b][:s_len]
        in_maps.append({
            "x": np.ascontiguousarray(x, np.float32),
            "wq": wq_p, "wk": wk_p, "wv": wv_p, "wo": wo_p, "bo": bo_t,
        })
    return in_maps


def _run(inputs, s_len=S, use_rs=USE_RS, trace=False, tmpdir=None):
    m1 = np.asarray(inputs["modal1_input"], np.float32)
    m2 = np.asarray(inputs["modal2_input"], np.float32)
    args = [np.asarray(inputs[k], np.float32)
            for k in ("Wq", "bq", "Wk", "bk", "Wv", "bv", "Wo", "bo")]
    nc = _build(s_len, use_rs)
    in_maps = _prep_in_maps(m1, m2, *args, s_len=s_len)
    kr = run_bass_kernel_spmd(nc, in_maps, core_ids=list(range(NCORES)),
                              trace=trace, tmpdir=tmpdir)
    res = kr.results
    half = s_len // 2
    out = np.empty((B, s_len, D), np.float32)
    for b in range(B):
        if use_rs:
            out[b, :half] = res[2 * b]["out"]
            out[b, half:] = res[2 * b + 1]["out"]
        else:
            out[b] = res[2 * b]["out"] + res[2 * b + 1]["out"]
    return out, kr


def kernel(**inputs):
    out, _ = _run(inputs)
    return out


# revision 7
# speedup vs baseline: 1.2938x; 1.2938x over previous
"""Trainium2 Bass kernel for AttentionFusionModel (B=4, S=4096, D=200).

out = (attn(x1) + attn(x2)) @ Wo.T + bo, with attn sharing Wq/Wk/Wv.

Sharding: 8 (batch, modality) pairs -> 8 NeuronCores, one full self-attention
per core. Modality fusion = pairwise ReduceScatter between cores (2b, 2b+1),
each core projecting its own attention output first (projection is linear, so
proj(a1 + a2) = proj(a1) + proj(a2); softmax row-normalization commutes with
the projection and is applied post-projection as a per-row scale). The RS is
chunked so it overlaps the tail of the attention compute.

Per-core layout strategy (all big matmuls in bf16, fp32 PSUM accumulate):
  X^T [201, S]   (d on partitions, +ones row)  via DMA-transpose (bf16 xbar)
  Q^T, K^T [200, S] = (W·sc)^T-stationary matmuls (bias via ones-row aug)
  V [S, 201]     natural layout, +ones column (for sumexp)
  scores^T[k,q] tiles = K^T-slice.T @ Q^T   (contract d: 128+72 blocks)
  expT = Exp(scores^T)  on ScalarE (no max subtraction; |scores| ~< 7)
  U^T[d+1, q] += V'[k,:].T @ expT           (row 200 = sumexp L)
  proj[q, 201] = U^T-slice.T @ Wo'^T        (col 200 = L passthrough)
  out rows = proj[:, :200] * (1/L) + bo/2
"""

import os
import sys

sys.path.insert(0, "/opt/trn_rl_repo")

import numpy as np
from contextlib import ExitStack

from concourse import bacc, mybir, tile
from concourse.bass_utils import run_bass_kernel_spmd
from concourse.masks import make_identity

F32 = mybir.dt.float32
BF16 = mybir.dt.bfloat16
AF = mybir.ActivationFunctionType
ALU = mybir.AluOpType

B = 4
S = 4096
D = 200
P = 128
D2 = D - P            # 72
DA = D + 1            # 201 (augmented with ones row / sumexp col)
NCORES = 8
RG = [[0, 1], [2, 3], [4, 5], [6, 7]]  # core 2b+m handles (batch b, modality m)

USE_RS = True


def _emit_av(nc, acc1, acc2, v_sb, et, kb, nkb):
    k0 = kb * DA
    st = kb == 0
    sp = kb == nkb - 1
    nc.tensor.matmul(acc1[:], v_sb[:, k0:k0 + P], et[:], start=st, stop=sp)
    nc.tensor.matmul(acc2[:], v_sb[:, k0 + P:k0 + DA], et[:], start=st, stop=sp)


def _emit(ctx, tc, nc, exts, s_len, use_rs):
    x_ext, wq_ext, wk_ext, wv_ext, wo_ext, bo_ext, out_ext, rs_bufs = exts
    QG = min(512, s_len)
    nkb = s_len // P
    nqg = s_len // QG
    qg_per_chunk = min(2, nqg)
    nchunk = nqg // qg_per_chunk
    crows = qg_per_chunk * QG  # rs chunk input rows

    pers = ctx.enter_context(tc.tile_pool(name="pers", bufs=1))
    xt0 = pers.tile([P, s_len], BF16)
    xt1 = pers.tile([P, s_len], BF16)       # rows 0:72 = d 128:200, 72 = ones
    qt0 = pers.tile([P, s_len], BF16)
    qt1 = pers.tile([D2, s_len], BF16)
    kt0 = pers.tile([P, s_len], BF16)
    kt1 = pers.tile([D2, s_len], BF16)
    v_sb = pers.tile([P, nkb * DA], BF16)   # nkb tiles of [128, 201], col 200 = 1s
    wq0 = pers.tile([P, D], BF16)
    wq1 = pers.tile([D2 + 1, D], BF16)
    wk0 = pers.tile([P, D], BF16)
    wk1 = pers.tile([D2 + 1, D], BF16)
    wv0 = pers.tile([P, DA], BF16)
    wv1 = pers.tile([D2 + 1, DA], BF16)
    wo0 = pers.tile([P, DA], BF16)
    wo1 = pers.tile([D2 + 1, DA], BF16)
    bo_sb = pers.tile([P, D], F32)

    # ---- phase 1: load X, build X^T via PE transposes ----
    ident = pers.tile([P, P], F32)
    make_identity(nc, ident[:])
    # ones row lives at partition 72 of xt1; engine APs need 32-aligned
    # partition bases, so memset [64:128) and let the X^T copies overwrite
    # rows 64..71 (rows 73.. stay harmless junk, never read).
    nc.vector.memset(xt1[64:P, :], 1.0)
    with ExitStack() as ph1:
        xp = ph1.enter_context(tc.tile_pool(name="xp", bufs=4))
        tps = ph1.enter_context(tc.tile_pool(name="tps", bufs=2, space="PSUM"))
        for n in range(nkb):
            c0, c1 = n * P, (n + 1) * P
            x_in = xp.tile([P, D], F32, tag="xin")
            nc.sync.dma_start(out=x_in[:], in_=x_ext[c0:c1, :])
            p1 = tps.tile([P, P], F32, tag="tp1")
            nc.tensor.transpose(p1[:], x_in[:, 0:P], ident[:])
            p2 = tps.tile([D2, P], F32, tag="tp2")
            nc.tensor.transpose(p2[:], x_in[:, P:D], ident[:])
            nc.vector.tensor_copy(xt0[:, c0:c1], p1[:])
            nc.vector.tensor_copy(xt1[0:D2, c0:c1], p2[:])

        nc.sync.dma_start(out=bo_sb[:], in_=bo_ext[:, :])
        wst = ph1.enter_context(tc.tile_pool(name="wstage", bufs=2))
        for (ext, b0, b1, width) in [
            (wq_ext, wq0, wq1, D),
            (wk_ext, wk0, wk1, D),
            (wv_ext, wv0, wv1, DA),
            (wo_ext, wo0, wo1, DA),
        ]:
            wf0 = wst.tile([P, DA], F32, tag="wf0")
            wf1 = wst.tile([D2 + 1, DA], F32, tag="wf1")
            nc.sync.dma_start(out=wf0[:, 0:width], in_=ext[0:P, :])
            nc.sync.dma_start(out=wf1[:, 0:width], in_=ext[P:DA, :])
            nc.vector.tensor_copy(b0[:, 0:width], wf0[:, 0:width])
            nc.vector.tensor_copy(b1[:, 0:width], wf1[:, 0:width])

        # ---- QKV projections ----
        qkps = ph1.enter_context(tc.tile_pool(name="qkps", bufs=2, space="PSUM"))
        CH = min(512, s_len)
        for (w0, w1, t0, t1) in [(wq0, wq1, qt0, qt1), (wk0, wk1, kt0, kt1)]:
            for ob, obw in [(0, P), (1, D2)]:
                tdst = t0 if ob == 0 else t1
                for ch in range(s_len // CH):
                    c0, c1 = ch * CH, (ch + 1) * CH
                    ps = qkps.tile([P, CH], F32, tag="qk")
                    nc.tensor.matmul(ps[0:obw, :], w0[:, ob * P:ob * P + obw],
                                     xt0[:, c0:c1], start=True, stop=False)
                    nc.tensor.matmul(ps[0:obw, :], w1[:, ob * P:ob * P + obw],
                                     xt1[0:D2 + 1, c0:c1], start=False, stop=True)
                    nc.vector.tensor_copy(tdst[:, c0:c1], ps[0:obw, :])

        vps = ph1.enter_context(tc.tile_pool(name="vps", bufs=2, space="PSUM"))
        for n in range(nkb):
            c0, c1 = n * P, (n + 1) * P
            pv = vps.tile([P, DA], F32, tag="pv")
            nc.tensor.matmul(pv[:], xt0[:, c0:c1], wv0[:], start=True, stop=False)
            nc.tensor.matmul(pv[:], xt1[0:D2 + 1, c0:c1], wv1[:],
                             start=False, stop=True)
            nc.vector.tensor_copy(v_sb[:, n * DA:(n + 1) * DA], pv[:])

    # ---- phase 2: attention + projection + epilogue (+ chunked RS) ----
    with ExitStack() as ph2:
        scp = ph2.enter_context(tc.tile_pool(name="scp", bufs=2, space="PSUM"))
        accp = ph2.enter_context(tc.tile_pool(name="accp", bufs=2, space="PSUM"))
        projp = ph2.enter_context(tc.tile_pool(name="projp", bufs=2, space="PSUM"))
        etp = ph2.enter_context(tc.tile_pool(name="etp", bufs=3))
        utp = ph2.enter_context(tc.tile_pool(name="utp", bufs=2))
        epip = ph2.enter_context(tc.tile_pool(name="epip", bufs=4))
        for qg in range(nqg):
            q0, q1 = qg * QG, (qg + 1) * QG
            acc1 = accp.tile([P, QG], F32, tag="acc1")
            acc2 = accp.tile([D2 + 1, QG], F32, tag="acc2")
            ets = {}
            for kb in range(nkb):
                k0 = kb * P
                sc_ps = scp.tile([P, QG], F32, tag="sc")
                nc.tensor.matmul(sc_ps[:], kt0[:, k0:k0 + P], qt0[:, q0:q1],
                                 start=True, stop=False)
                nc.tensor.matmul(sc_ps[:], kt1[:, k0:k0 + P], qt1[:, q0:q1],
                                 start=False, stop=True)
                et = etp.tile([P, QG], BF16, tag="et")
                nc.scalar.activation(et[:], sc_ps[:], AF.Exp)
                ets[kb] = et
                if kb >= 1:
                    _emit_av(nc, acc1, acc2, v_sb, ets.pop(kb - 1), kb - 1, nkb)
            _emit_av(nc, acc1, acc2, v_sb, ets.pop(nkb - 1), nkb - 1, nkb)

            ut0 = utp.tile([P, QG], BF16, tag="ut0")
            ut1 = utp.tile([D2 + 1, QG], BF16, tag="ut1")
            nc.vector.tensor_copy(ut0[:], acc1[:])
            nc.vector.tensor_copy(ut1[:], acc2[:])
            chunk = qg // qg_per_chunk
            for qb in range(QG // P):
                pp = projp.tile([P, DA], F32, tag="pp")
                nc.tensor.matmul(pp[:], ut0[:, qb * P:(qb + 1) * P], wo0[:],
                                 start=True, stop=False)
                nc.tensor.matmul(pp[:], ut1[:, qb * P:(qb + 1) * P], wo1[:],
                                 start=False, stop=True)
                rc = epip.tile([P, 1], F32, tag="rc")
                nc.vector.reciprocal(rc[:], pp[:, D:DA])
                ot = epip.tile([P, D], F32, tag="ot")
                nc.vector.tensor_scalar(ot[:], pp[:, 0:D], rc[:], None, ALU.mult)
                nc.vector.tensor_tensor(ot[:], ot[:], bo_sb[:], ALU.add)
                r0 = q0 + qb * P
                if use_rs:
                    dst = rs_bufs[chunk][0]
                    nc.sync.dma_start(
                        out=dst[r0 - chunk * crows:r0 - chunk * crows + P, :],
                        in_=ot[:])
                else:
                    nc.sync.dma_start(out=out_ext[r0:r0 + P, :], in_=ot[:])

            if use_rs and (qg + 1) % qg_per_chunk == 0:
                ci, co = rs_bufs[chunk]
                nc.gpsimd.collective_compute(
                    "ReduceScatter",
                    ALU.add,
                    replica_groups=RG,
                    ins=[ci[:, :].opt()],
                    outs=[co[:, :].opt()],
                )
                orow = chunk * (crows // 2)
                nc.sync.dma_start(out=out_ext[orow:orow + crows // 2, :],
                                  in_=co[:, :])


_CACHE = {}


def _build(s_len=S, use_rs=USE_RS):
    key = (s_len, use_rs)
    if key not in _CACHE:
        nc = bacc.Bacc("TRN2", target_bir_lowering=False, debug=False,
                       num_devices=NCORES)
        x_ext = nc.dram_tensor("x", [s_len, D], F32, kind="ExternalInput")
        wq_ext = nc.dram_tensor("wq", [DA, D], F32, kind="ExternalInput")
        wk_ext = nc.dram_tensor("wk", [DA, D], F32, kind="ExternalInput")
        wv_ext = nc.dram_tensor("wv", [DA, DA], F32, kind="ExternalInput")
        wo_ext = nc.dram_tensor("wo", [DA, DA], F32, kind="ExternalInput")
        bo_ext = nc.dram_tensor("bo", [P, D], F32, kind="ExternalInput")
        out_rows = s_len // 2 if use_rs else s_len
        out_ext = nc.dram_tensor("out", [out_rows, D], F32, kind="ExternalOutput")
        rs_bufs = []
        if use_rs:
            QG = min(512, s_len)
            nqg = s_len // QG
            qg_per_chunk = min(2, nqg)
            nchunk = nqg // qg_per_chunk
            crows = qg_per_chunk * QG
            for g in range(nchunk):
                ci = nc.dram_tensor(f"rs_in{g}", [crows, D], F32)
                co = nc.dram_tensor(f"rs_out{g}", [crows // 2, D], F32)
                rs_bufs.append((ci, co))
        exts = (x_ext, wq_ext, wk_ext, wv_ext, wo_ext, bo_ext, out_ext, rs_bufs)
        with tile.TileContext(nc) as tc:
            with ExitStack() as ctx:
                _emit(ctx, tc, nc, exts, s_len, use_rs)
        nc.compile()
        _CACHE[key] = nc
    return _CACHE[key]


def _prep_in_maps(m1, m2, Wq, bq, Wk, bk, Wv, bv, Wo, bo, s_len=S):
    sc = np.float32(1.0 / np.sqrt(D))
    wq_p = np.zeros((DA, D), np.float32)
    wq_p[:D] = Wq.T * sc
    wq_p[D] = bq * sc
    wk_p = np.zeros((DA, D), np.float32)
    wk_p[:D] = Wk.T
    wk_p[D] = bk
    wv_p = np.zeros((DA, DA), np.float32)
    wv_p[:D, :D] = Wv.T
    wv_p[D, :D] = bv
    wv_p[D, D] = 1.0
    wo_p = np.zeros((DA, DA), np.float32)
    wo_p[:D, :D] = Wo.T
    wo_p[D, D] = 1.0
    bo_t = np.ascontiguousarray(
        np.broadcast_to((bo * 0.5).astype(np.float32), (P, D)))
    in_maps = []
    for c in range(NCORES):
        b, m = c // 2, c % 2
        x = (m1 if m == 0 else m2)[b][:s_len]
        in_maps.append({
            "x": np.ascontiguousarray(x, np.float32),
            "wq": wq_p, "wk": wk_p, "wv": wv_p, "wo": wo_p, "bo": bo_t,
        })
    return in_maps


def _run(inputs, s_len=S, use_rs=USE_RS, trace=False, tmpdir=None):
    m1 = np.asarray(inputs["modal1_input"], np.float32)
    m2 = np.asarray(inputs["modal2_input"], np.float32)
    args = [np.asarray(inputs[k], np.float32)
            for k in ("Wq", "bq", "Wk", "bk", "Wv", "bv", "Wo", "bo")]
    nc = _build(s_len, use_rs)
    in_maps = _prep_in_maps(m1, m2, *args, s_len=s_len)
    kr = run_bass_kernel_spmd(nc, in_maps, core_ids=list(range(NCORES)),
                              trace=trace, tmpdir=tmpdir)
    res = kr.results
    out = np.empty((B, s_len, D), np.float32)
    if use_rs:
        # chunked RS: core 2b holds the first half of every chunk, core 2b+1
        # the second half; chunk g covers global rows [g*crows, (g+1)*crows)
        QG = min(512, s_len)
        nqg = s_len // QG
        crows = min(2, nqg) * QG
        csz = crows // 2
        nchunk = s_len // crows
        for b in range(B):
            for g in range(nchunk):
                lo, hi = g * csz, (g + 1) * csz
                out[b, g * crows:g * crows + csz] = res[2 * b]["out"][lo:hi]
                out[b, g * crows + csz:(g + 1) * crows] = \
                    res[2 * b + 1]["out"][lo:hi]
    else:
        for b in range(B):
            out[b] = res[2 * b]["out"] + res[2 * b + 1]["out"]
    return out, kr


def kernel(**inputs):
    out, _ = _run(inputs)
    return out
